# revision 36
# baseline (speedup 1.0000x reference)
"""Bidirectional Vim-Mamba2 encoder on 8 Trainium2 NeuronCores.

Sharding: core c -> (sample b = c//2, direction d = c%2). Each core runs the
full per-sample block chain for its direction (chunked-SSD form of the
selective scan); the bidirectional combine after layer 1 is a pairwise
AllGather + on-chip average. Direction needs no data reversal on device:
bwd cores get chunk-reversed input (host side), anticausal masks / suffix
cumsums via per-core constant data, and a dynamic conv tap offset.

All big GEMMs run bf16 x bf16 (weights converted host-side, activations
produced in bf16); SSM state and residual stream stay fp32.

Scan decay broadcasts: per chunk the fp32 decay rows L[h, :] are flattened
h-major into a [1, 2048] row (one DMA) and broadcast across partitions by a
rank-1 f32r matmul; the per-token bias (-L_j + ln dt_j) rides a second
accumulating matmul against a block-select 0/1 constant, so exp() runs as
four wide activations per chunk with no per-head bias columns.

Self-contained: hardcodes shapes; imports concourse from known paths.
"""
import os
import sys

for _p in ("/opt/trn_rl_repo", "/root/.axon_site/_ro/trn_rl_repo"):
    if _p not in sys.path:
        sys.path.append(_p)

import numpy as np
import ml_dtypes
import concourse.bass as bass
import concourse.bacc as bacc
import concourse.mybir as mybir
from concourse import tile
from concourse.bass_utils import run_bass_kernel_spmd
from concourse.ap import AP


def _rev(ap, n=128):
    # reverse the innermost (contiguous) free axis of an AP
    a = [list(d) for d in ap.ap]
    assert a[-1][0] == 1 and a[-1][1] == n, a
    return AP(ap.tensor, ap.offset + n - 1, a[:-1] + [[-1, n]])

F32 = mybir.dt.float32
F32R = mybir.dt.float32r
BF16 = mybir.dt.bfloat16
I32 = mybir.dt.int32
AF = mybir.ActivationFunctionType
OP = mybir.AluOpType

T, D = 1024, 512
H, Q, NCH = 16, 128, 8
D_INNER = 1024
D_INPROJ = 2192
MPAD = 2304
FFN = 2048
EPS = 1e-5
FMAX = 3.0e38
BF = ml_dtypes.bfloat16

_CACHE = {}
PH = int(os.environ.get("KPH", "99"))


def build_nc():
    nc = bacc.Bacc("TRN2", target_bir_lowering=False, debug=False, num_devices=8)

    x_in = nc.dram_tensor("x_fm", [D, T], F32, kind="ExternalInput")
    wip = nc.dram_tensor("wip", [2, 4, 128, MPAD], BF16, kind="ExternalInput")
    wop = nc.dram_tensor("wop", [2, 8, 128, D], BF16, kind="ExternalInput")
    wf1 = nc.dram_tensor("wf1", [2, 4, 128, FFN], BF16, kind="ExternalInput")
    wf2 = nc.dram_tensor("wf2", [2, 16, 128, D], BF16, kind="ExternalInput")
    bf1 = nc.dram_tensor("bf1", [2, 128, 16], F32, kind="ExternalInput")
    bf2 = nc.dram_tensor("bf2", [2, 128, 4], F32, kind="ExternalInput")
    wcv = nc.dram_tensor("wcv", [2, 128, 63], F32, kind="ExternalInput")
    bcv = nc.dram_tensor("bcv", [2, 128, 9], F32, kind="ExternalInput")
    lnw = nc.dram_tensor("lnw", [2, 2, 128, 4], F32, kind="ExternalInput")
    gwc = nc.dram_tensor("gwc", [2, 128, 8], F32, kind="ExternalInput")
    dcol = nc.dram_tensor("dcol", [2, 128, 8], F32, kind="ExternalInput")
    dtbt = nc.dram_tensor("dtbt", [32, 2], F32, kind="ExternalInput")
    neat = nc.dram_tensor("neat", [32, 2], F32, kind="ExternalInput")
    causal = nc.dram_tensor("causal", [128, 128], F32, kind="ExternalInput")
    identh = nc.dram_tensor("identh", [128, 128], BF16, kind="ExternalInput")
    onesh = nc.dram_tensor("onesh", [1, 128], F32, kind="ExternalInput")
    onescol = nc.dram_tensor("onescol", [128, 1], F32, kind="ExternalInput")
    selcol = nc.dram_tensor("selcol", [128, 2], F32, kind="ExternalInput")
    onesdh = nc.dram_tensor("onesdh", [32, 128], F32, kind="ExternalInput")
    epsh = nc.dram_tensor("epsh", [1, 1], F32, kind="ExternalInput")
    i16eh = nc.dram_tensor("i16eh", [48, 2048], F32R, kind="ExternalInput")
    id32h = nc.dram_tensor("id32h", [32, 32], F32, kind="ExternalInput")
    idselh = nc.dram_tensor("idselh", [48, 32], F32R, kind="ExternalInput")
    sel8h = nc.dram_tensor("sel8h", [8, 512], F32R, kind="ExternalInput")
    out_t = nc.dram_tensor("out_fm", [D, T], F32, kind="ExternalOutput")

    with tile.TileContext(nc) as tc:
        _emit(nc, tc, locals())
    nc.compile()
    return nc


def _emit(nc, tc, t_):
    x_in, out_t = t_["x_in"], t_["out_t"]
    wip, wop, wf1, wf2 = t_["wip"], t_["wop"], t_["wf1"], t_["wf2"]

    const = tc.alloc_tile_pool(name="const", bufs=1)
    dram = tc.alloc_tile_pool(name="dram", bufs=1, space="DRAM")

    # ---------------- constants ----------------
    def load_const(name, shape, src_ap, dtype=F32):
        t = const.tile(shape, dtype, name=name)
        nc.sync.dma_start(t[:], src_ap)
        return t

    x_res = [const.tile([128, T], F32, name=f"xres{i}") for i in range(4)]
    for i in range(4):
        nc.sync.dma_start(x_res[i][:], x_in.ap()[128 * i:128 * (i + 1), :])

    causal_t = load_const("causal_t", [128, 128], t_["causal"].ap())
    ident_b = const.tile([128, 128], BF16, name="ident_b")
    nc.sync.dma_start(ident_b[:], t_["identh"].ap())
    ones_f = load_const("ones_f", [1, 128], t_["onesh"].ap())
    ones_r = const.tile([1, 128], F32R, name="ones_r")
    nc.vector.tensor_copy(ones_r[:], ones_f[:])
    onesP = const.tile([128, 128], F32R, name="onesP")
    nc.vector.memset(onesP.bitcast(F32), 1.0)
    onescol_f = load_const("onescol_f", [128, 1], t_["onescol"].ap())
    onescol_b = const.tile([128, 1], BF16, name="onescol_b")
    nc.vector.tensor_copy(onescol_b[:], onescol_f[:])
    i16e_sb = load_const("i16e_sb", [48, 2048], t_["i16eh"].ap(), dtype=F32R)
    id32f = load_const("id32f", [32, 32], t_["id32h"].ap())
    idsel = load_const("idsel", [48, 32], t_["idselh"].ap(), dtype=F32R)
    sel8_sb = load_const("sel8_sb", [8, 512], t_["sel8h"].ap(), dtype=F32R)
    dtb_t = load_const("dtb_t", [32, 2], t_["dtbt"].ap())
    nea_t = load_const("nea_t", [32, 2], t_["neat"].ap())
    lnw_sb = const.tile([128, 16], F32, name="lnw_sb")
    nc.sync.dma_start(lnw_sb.rearrange("p (a b c) -> p a b c", a=2, b=2),
                      t_["lnw"].ap().rearrange("a b p c -> p a b c"))
    gwc_sb = const.tile([128, 16], F32, name="gwc_sb")
    nc.sync.dma_start(gwc_sb.rearrange("p (a c) -> p a c", a=2),
                      t_["gwc"].ap().rearrange("a p c -> p a c"))
    dcol_sb = const.tile([128, 16], F32, name="dcol_sb")
    nc.sync.dma_start(dcol_sb.rearrange("p (a c) -> p a c", a=2),
                      t_["dcol"].ap().rearrange("a p c -> p a c"))
    wcv_sb = const.tile([128, 126], F32, name="wcv_sb")
    nc.sync.dma_start(wcv_sb.rearrange("p (a c) -> p a c", a=2),
                      t_["wcv"].ap().rearrange("a p c -> p a c"))
    bcv_sb = const.tile([128, 18], F32, name="bcv_sb")
    nc.sync.dma_start(bcv_sb.rearrange("p (a c) -> p a c", a=2),
                      t_["bcv"].ap().rearrange("a p c -> p a c"))
    bf1_sb = const.tile([128, 32], F32, name="bf1_sb")
    nc.sync.dma_start(bf1_sb.rearrange("p (a c) -> p a c", a=2),
                      t_["bf1"].ap().rearrange("a p c -> p a c"))
    bf2_sb = const.tile([128, 8], F32, name="bf2_sb")
    nc.sync.dma_start(bf2_sb.rearrange("p (a c) -> p a c", a=2),
                      t_["bf2"].ap().rearrange("a p c -> p a c"))
    sel_t = load_const("sel_t", [128, 2], t_["selcol"].ap())

    eps1 = load_const("eps1", [1, 1], t_["epsh"].ap())
    onesd_c = load_const("onesd_c", [32, 128], t_["onesdh"].ap())
    hT = const.tile([64, 1024], F32R, name="hT")

    # preloaded bf16 weights: layer 0 all + layer 1 in_proj at kernel start;
    # layer 1 out_proj/FFN staged at layer-1 start (SBUF headroom)
    pwA = tc.alloc_tile_pool(name="pwA", bufs=1, side="left")
    wopb0 = [pwA.tile([128, D], BF16, name=f"wopb0{k}") for k in range(8)]
    wf1b0 = [pwA.tile([128, FFN], BF16, name=f"wf1b0{k}") for k in range(4)]
    wf2b0 = [pwA.tile([128, D], BF16, name=f"wf2b0{k}") for k in range(16)]
    for k in range(8):
        nc.sync.dma_start(wopb0[k][:], wop.ap()[0, k])
    for k in range(4):
        nc.sync.dma_start(wf1b0[k][:], wf1.ap()[0, k])
    for k in range(16):
        nc.sync.dma_start(wf2b0[k][:], wf2.ap()[0, k])

    cc_in = dram.tile([D, T], BF16, name="cc_in")
    cc_out = [dram.tile([2, 128, T], BF16, name=f"cc_out{i}") for i in range(4)]

    ones_c1 = onescol_b[:, 0:1]   # [K=128, M=1] for partition-sum matmuls

    def _ln(l, which, out_pool):
        """feature-dim LayerNorm of x_res -> 4 bf16 tiles in out_pool."""
        w_col = lnw_sb[:, (l * 2 + which) * 4:(l * 2 + which) * 4 + 4]
        outs = [out_pool.tile([128, T], BF16, tag=f"hln{i}", name=f"hln{i}")
                for i in range(4)]
        px = tc.alloc_tile_pool(name=f"ln{l}{which}x", bufs=2)
        pp = tc.alloc_tile_pool(name=f"ln{l}{which}p", bufs=2, space="PSUM")
        xr, x2 = [], []
        for i in range(4):
            a = px.tile([128, T], BF16, tag="xr", bufs=4, name=f"xr{i}")
            nc.vector.tensor_copy(a[:], x_res[i][:])
            b = px.tile([128, T], BF16, tag="x2", bufs=4, name=f"x2{i}")
            nc.scalar.activation(b[:], x_res[i][:], AF.Square)
            xr.append(a)
            x2.append(b)
        m_row = px.tile([1, T], F32R, bufs=1, name="m_row")
        r_rowr = px.tile([1, T], F32R, bufs=1, name="r_rowr")
        for tb in range(2):
            sl = slice(512 * tb, 512 * (tb + 1))
            ps = pp.tile([1, 512], F32, tag="st", name="st")
            for k in range(4):
                nc.tensor.matmul(ps[:], ones_c1, xr[k][:, sl],
                                 start=(k == 0), stop=(k == 3))
            ps2 = pp.tile([1, 512], F32, tag="st2", name="st2")
            for k in range(4):
                nc.tensor.matmul(ps2[:], ones_c1, x2[k][:, sl],
                                 start=(k == 0), stop=(k == 3))
            nc.scalar.activation(m_row[0:1, sl], ps[:], AF.Copy, scale=1.0 / D)
            msq = px.tile([1, 512], F32, tag="msq", name="msq")
            nc.scalar.activation(msq[:], m_row[0:1, sl], AF.Square)
            var = px.tile([1, 512], F32, tag="var", name="var")
            nc.vector.scalar_tensor_tensor(out=var[:], in0=ps2[:],
                                           scalar=1.0 / D, in1=msq[:],
                                           op0=OP.mult, op1=OP.subtract)
            # 1/sqrt(v+eps) = exp(-0.5*ln(v+eps)): scalar-engine only, avoids
            # the slow DVE reciprocal (ln+exp share one act table set)
            lnv = px.tile([1, 512], F32, tag="lnv", name="lnv")
            nc.scalar.activation(lnv[:], var[:], AF.Ln, bias=eps1[:])
            nc.scalar.activation(r_rowr[0:1, sl], lnv[:], AF.Exp, scale=-0.5)
        for i in range(4):
            for tb in range(2):
                sl = slice(512 * tb, 512 * (tb + 1))
                mb = pp.tile([128, 512], F32, tag="mb", name="mb")
                nc.tensor.matmul(mb[:], ones_r[0:1, :], m_row[0:1, sl],
                                 start=True, stop=True)
                rb = pp.tile([128, 512], F32, tag="rb", name="rb")
                nc.tensor.matmul(rb[:], ones_r[0:1, :], r_rowr[0:1, sl],
                                 start=True, stop=True)
                tmp = px.tile([128, 512], F32, tag="tmp", name="tmp")
                nc.vector.tensor_tensor(out=tmp[:], in0=x_res[i][:, sl],
                                        in1=mb[:], op=OP.subtract)
                nc.vector.scalar_tensor_tensor(
                    out=outs[i][:, sl], in0=tmp[:], scalar=w_col[:, i:i + 1],
                    in1=rb[:], op0=OP.mult, op1=OP.mult)
        pp.release()
        px.release()
        return outs

    # ======================= layers =======================
    for l in range(1 if PH < 10 else 2):
        # layer-1 out_proj/FFN weights staged at layer start (DMA overlaps
        # the mixer phases)
        if l == 1:
            pwA.release()
            pwA = None
            pwB = tc.alloc_tile_pool(name="pwB", bufs=1, side="left")
            wopb = [pwB.tile([128, D], BF16, name=f"wopb1{k}")
                    for k in range(8)]
            wf1b = [pwB.tile([128, FFN], BF16, name=f"wf1b1{k}")
                    for k in range(4)]
            wf2b = [pwB.tile([128, D], BF16, name=f"wf2b1{k}")
                    for k in range(16)]
            for k in range(8):
                nc.sync.dma_start(wopb[k][:], wop.ap()[1, k])
            for k in range(4):
                nc.sync.dma_start(wf1b[k][:], wf1.ap()[1, k])
            for k in range(16):
                nc.sync.dma_start(wf2b[k][:], wf2.ap()[1, k])
        else:
            wopb, wf1b, wf2b = wopb0, wf1b0, wf2b0

        # two-sided stack pool management (LIFO per side)
        pCz = tc.alloc_tile_pool(name=f"Cz{l}", bufs=1, side="left")
        pF1 = tc.alloc_tile_pool(name=f"F1{l}", bufs=1, side="right")
        pA = tc.alloc_tile_pool(name=f"A{l}", bufs=1, side="right")
        # in_proj weights live on the right stack, released after in_proj
        # so the scan phase gets the SBUF back
        pwI = tc.alloc_tile_pool(name=f"pwI{l}", bufs=1, side="right")
        wipb = [pwI.tile([128, MPAD], BF16, name=f"wipb{l}{k}")
                for k in range(4)]
        for k in range(4):
            nc.sync.dma_start(wipb[k][:], wip.ap()[l, k])
        pH = tc.alloc_tile_pool(name=f"H{l}", bufs=1, side="left")

        z_t = [pCz.tile([128, T], BF16, tag=f"zt{i}", name=f"z{i}")
               for i in range(8)]
        xpad = [pA.tile([128, 1028], BF16, tag=f"xpad{f}", name=f"xpad{f}")
                for f in range(9)]
        dtr = pF1.tile([32, T], F32, name="dtr")
        nc.vector.memset(dtr[:], 0.0)

        if PH <= 0:
            pH.release()
            pwI.release()
            pA.release()
            pCz.release()
            pF1.release()
            break
        with nc.named_scope(f"L{l}.ln1"):
            h_ln1 = _ln(l, 0, pH)
        if PH <= 1:
            pH.release()
            pwI.release()
            pA.release()
            pCz.release()
            pF1.release()
            break

        # ---------------- in_proj ----------------
        with nc.named_scope(f"L{l}.inproj"):
            pp = tc.alloc_tile_pool(name=f"ipp{l}", bufs=3, space="PSUM")
            for mt in range(18):
                mo = 128 * mt
                for tb in range(2):
                    sl = slice(512 * tb, 512 * (tb + 1))
                    ps = pp.tile([128, 512], F32, tag="mm", name="ps")
                    for k in range(4):
                        nc.tensor.matmul(ps[:], wipb[k][:, mo:mo + 128],
                                         h_ln1[k][:, sl],
                                         start=(k == 0), stop=(k == 3))
                    if mt < 8:
                        nc.scalar.activation(z_t[mt][:, sl], ps[:], AF.Silu)
                    elif mt < 17:
                        f = mt - 8
                        nc.scalar.activation(
                            xpad[f][:, 3 + 512 * tb:3 + 512 * (tb + 1)], ps[:],
                            AF.Copy)
                    else:
                        nc.vector.tensor_copy(dtr[0:16, sl], ps[0:16, :])
            pp.release()
        pwI.release()
        pH.release()
        if PH <= 2:
            pA.release()
            pCz.release()
            pF1.release()
            break

        # ---------------- conv + silu (bf16) ----------------
        pB = tc.alloc_tile_pool(name=f"B{l}", bufs=1, side="left")
        xsil = [pB.tile([128, T], BF16, tag=f"xsil{f}", name=f"xsil{f}")
                for f in range(8)]
        B_t = pB.tile([64, T], BF16, name="B_t")
        C_t = pB.tile([64, T], BF16, name="C_t")
        with nc.named_scope(f"L{l}.conv"):
            # depthwise causal conv on the PE: stationary = diag(w_k) built
            # on the fly from the bf16 identity, moving = shifted x slices,
            # 4 taps accumulate in PSUM, silu reads PSUM
            pcv = tc.alloc_tile_pool(name=f"cv{l}", bufs=2, side="right")
            pcp = tc.alloc_tile_pool(name=f"cvp{l}", bufs=2, space="PSUM")
            for f in range(9):
                xp = xpad[f]
                nc.vector.memset(xp[:, 0:3], 0.0)
                dg = pcv.tile([128, 4, 128], BF16, tag="dg", name="dg")
                for k in range(4):
                    w_k = wcv_sb[:, l * 63 + f * 7 + k:l * 63 + f * 7 + k + 1]
                    nc.vector.tensor_scalar(
                        out=dg[:, k], in0=ident_b[:, :], scalar1=w_k,
                        scalar2=None, op0=OP.mult)
                for tb in range(2):
                    acc = pcp.tile([128, 512], F32, tag="acc", name="acc")
                    for k in range(4):
                        nc.tensor.matmul(
                            acc[:], dg[:, k],
                            xp[:, k + 512 * tb:k + 512 * tb + 512],
                            start=(k == 0), stop=(k == 3))
                    sl2 = slice(512 * tb, 512 * (tb + 1))
                    if f < 8:
                        nc.scalar.activation(
                            xsil[f][:, sl2], acc[:], AF.Silu,
                            bias=bcv_sb[:, l * 9 + f:l * 9 + f + 1])
                    else:
                        nc.scalar.activation(
                            B_t[:, sl2], acc[0:64, :], AF.Silu,
                            bias=bcv_sb[0:64, l * 9 + f:l * 9 + f + 1])
                        nc.scalar.activation(
                            C_t[:, sl2], acc[64:128, :], AF.Silu,
                            bias=bcv_sb[64:128, l * 9 + f:l * 9 + f + 1])
            pcp.release()
            pcv.release()
        pA.release()
        if PH == 3:
            nc.vector.tensor_copy(x_res[0][:], xsil[0][:])
            nc.vector.tensor_copy(x_res[1][:], xsil[1][:])
            nc.vector.memset(x_res[2][:], 0.0)
            nc.vector.memset(x_res[3][:], 0.0)
            nc.vector.tensor_copy(x_res[2][0:64, :], B_t[:])
            nc.vector.tensor_copy(x_res[3][0:64, :], C_t[:])
        if PH <= 3:
            pB.release()
            pCz.release()
            pF1.release()
            break

        # ---------------- dt / decay family, St, B_tok ----------------
        pCy = tc.alloc_tile_pool(name=f"Cy{l}", bufs=1, side="right")
        y_t = [pCy.tile([128, T], BF16, tag=f"yt{i}", name=f"y{i}")
               for i in range(8)]
        pF2 = tc.alloc_tile_pool(name=f"F2{l}", bufs=1, side="right")
        pF3 = tc.alloc_tile_pool(name=f"F3{l}", bufs=1, side="right")
        dt_t = pF3.tile([32, T], F32, name="dt_t")
        log_a = pF3.tile([32, T], F32, name="log_a")
        gam_bc = pF2.tile([64, NCH * 16], F32, name="gam_bc")
        w_all = pF2.tile([128, NCH * 16], BF16, name="w_all")
        bfm_all = pF2.tile([32, NCH * 128], F32, name="bfm_all")
        Lf = pF2.tile([32, T], F32, name="Lf")
        # f32r hi/lo pairs (rows 0:16 hi, 32:48 lo residual, 16:32 zeroed)
        # so PE matmuls reconstruct full fp32 through two-row sums
        Lhl = pF2.tile([64, T], F32R, name="Lhl")
        bfm_hl = pF2.tile([64, T], F32R, name="bfm_hl")
        nc.vector.memset(Lhl.bitcast(F32)[0:32, :], 0.0)
        nc.vector.memset(bfm_hl.bitcast(F32)[0:32, :], 0.0)
        # decay-row broadcast staging: 8 chunk slots of (hi, lo) partition
        # pairs at bases {0,32,64,96} x 2 column slots, filled in the dt
        # phase so the scan's matmuls never wait on the sync queue
        Lrow = pF2.tile([128, 4096], F32R, name="Lrow")

        St_all = [pF2.tile([128, 128], BF16, tag=f"st{c}", name=f"St{c}")
                  for c in range(NCH)]
        B_tok = [pF2.tile([128, 64], BF16, tag=f"bt{c}", name=f"Bt{c}")
                 for c in range(NCH)]

        with nc.named_scope(f"L{l}.dtfam"):
            e_sp = pF3.tile([32, T], F32, name="e_sp")
            nc.scalar.activation(e_sp[:], dtr[:], AF.Exp, bias=dtb_t[:, l:l + 1])
            nc.scalar.activation(dt_t[:], e_sp[:], AF.Ln, bias=1.0)
            nc.vector.tensor_scalar(out=log_a[:], in0=dt_t[:],
                                    scalar1=nea_t[:, l:l + 1], scalar2=None,
                                    op0=OP.mult)

            pd = tc.alloc_tile_pool(name=f"dtf{l}", bufs=4, side="right")
            pg = tc.alloc_tile_pool(name=f"dtp{l}", bufs=2, space="PSUM")
            for c in range(NCH):
                sl = slice(Q * c, Q * (c + 1))
                la = log_a[:, sl]
                nc.vector.tensor_tensor_scan(Lf[:, sl], onesd_c[:], la, 0.0,
                                             OP.mult, OP.add)
                lndt = pd.tile([32, 128], F32, tag="lndt", name="lndt")
                nc.scalar.activation(lndt[:], dt_t[:, sl], AF.Ln)
                nc.vector.tensor_tensor(out=bfm_all[:, sl], in0=lndt[:],
                                        in1=Lf[:, sl], op=OP.subtract)
                # f32r hi/lo splits (rounding copy + residual)
                nc.vector.tensor_copy(Lhl[0:16, sl], Lf[0:16, sl])
                nc.vector.tensor_tensor(out=Lhl[32:48, sl], in0=Lf[0:16, sl],
                                        in1=Lhl[0:16, sl], op=OP.subtract)
                nc.vector.tensor_copy(bfm_hl[0:16, sl], bfm_all[0:16, sl])
                nc.vector.tensor_tensor(out=bfm_hl[32:48, sl],
                                        in0=bfm_all[0:16, sl],
                                        in1=bfm_hl[0:16, sl], op=OP.subtract)
                # flatten this chunk's L hi/lo rows h-major into its Lrow slot
                pb = 2 * (c % 4)
                co = 2048 * (c // 4)
                nc.sync.dma_start(
                    Lrow[pb:pb + 1, co:co + 2048].rearrange(
                        "p (h m) -> p h m", h=16), Lhl[0:16, sl])
                nc.sync.dma_start(
                    Lrow[pb + 1:pb + 2, co:co + 2048].rearrange(
                        "p (h m) -> p h m", h=16), Lhl[32:48, sl])
                # Lq (chunk-total log decay per head): select L[:, last token]
                # as a row via a tiny PE matmul (hi+lo summed by idsel)
                lqp = pg.tile([1, 32], F32, tag="lqp", name="lqp")
                nc.tensor.matmul(lqp[:], Lhl[0:48, Q * c + 127:Q * c + 128],
                                 idsel[:], start=True, stop=True)
                lqs = pd.tile([1, 32], F32, tag="lqs", name="lqs")
                nc.vector.tensor_copy(lqs[:], lqp[:])
                lqg = pd.tile([128, 16], F32, tag="lqg", name="lqg")
                nc.gpsimd.partition_broadcast(lqg[:], lqs[0:1, 0:16])
                nc.scalar.activation(gam_bc[:, 16 * c:16 * (c + 1)],
                                     lqg[0:64, :], AF.Exp)
                # w[j,h] = exp(Lq_h - L_j + lndt_j); bias columns via PE
                # transpose of bfm
                wtp = pg.tile([128, 32], F32, tag="wtp", name="wtp")
                nc.tensor.transpose(wtp[:], bfm_all[:, sl], id32f[:])
                wpre = pd.tile([128, 16], F32, tag="wpre", name="wpre")
                nc.vector.tensor_tensor(out=wpre[:], in0=wtp[:, 0:16],
                                        in1=lqg[:], op=OP.add)
                nc.scalar.activation(w_all[:, 16 * c:16 * (c + 1)], wpre[:],
                                     AF.Exp)
                stp = pg.tile([128, 128], F32, tag="stp", name="stp")
                nc.tensor.matmul(stp[:], B_t[:, sl], C_t[:, sl],
                                 start=True, stop=True)
                nc.vector.tensor_tensor(out=St_all[c][:], in0=stp[:],
                                        in1=causal_t[:], op=OP.mult)
                btp = pg.tile([128, 64], BF16, tag="btp", name="btp")
                nc.tensor.transpose(btp[:], B_t[:, sl], ident_b[0:64, 0:64])
                nc.vector.tensor_copy(B_tok[c][:], btp[:])
            pg.release()
            pd.release()
        pF3.release()
        if PH == 4:
            nc.vector.memset(x_res[0][:], 0.0)
            nc.vector.memset(x_res[1][:], 0.0)
            nc.vector.memset(x_res[3][:], 0.0)
            nc.vector.tensor_copy(x_res[0][0:32, :], Lf[:])
            nc.vector.tensor_copy(x_res[1][0:64, 0:128], gam_bc[:])
            nc.vector.tensor_copy(x_res[1][:, 128:256], w_all[:])
            nc.vector.tensor_copy(x_res[2][0:32, :], dt_t[:])
            for cc in range(8):
                nc.vector.tensor_copy(x_res[3][:, 128 * cc:128 * (cc + 1)],
                                      St_all[cc][:])
        if PH <= 4:
            pF2.release()
            pCy.release()
            pB.release()
            pCz.release()
            pF1.release()
            break

        # ---------------- scan ----------------
        nc.vector.memset(hT.bitcast(F32), 0.0)
        with nc.named_scope(f"L{l}.scan"):
            psc = tc.alloc_tile_pool(name=f"sc{l}", bufs=2, side="left")
            pbc = tc.alloc_tile_pool(name=f"bcp{l}", bufs=2, space="PSUM")
            psp = tc.alloc_tile_pool(name=f"spp{l}", bufs=1, space="PSUM")
            pxp = tc.alloc_tile_pool(name=f"xtp{l}", bufs=1, space="PSUM")
            for c in range(NCH):
                sl = slice(Q * c, Q * (c + 1))
                cm = 128 * (c % 4)
                co = 2048 * (c // 4)
                # x transposed: [token, (head, p)] in bf16
                xps = pxp.tile([128, 1024], BF16, tag="xps", name="xps")
                for f in range(8):
                    nc.tensor.transpose(xps[:, 128 * f:128 * (f + 1)],
                                        xsil[f][:, sl], ident_b[:, :])
                xtk = psc.tile([128, 1024], BF16, tag="xtk", name="xtk")
                nc.scalar.activation(xtk[:], xps[:], AF.Copy)
                hTb = psc.tile([64, 1024], BF16, tag="hTb", bufs=1, name="hTb")
                nc.scalar.activation(hTb[:], hT[:], AF.Copy)
                # within-chunk kernel: mexp[j,(h,m)] = exp(L[h,m] - L[h,j]
                # + lndt[h,j]); the L broadcast and the per-token bias both
                # ride the PE in fp32 (f32r hi/lo), so exp runs as 4 wide acts
                mexp = psc.tile([128, 16, 128], BF16, tag="mexp", name="mexp")
                mst = psc.tile([64, 16, 128], BF16, tag="mst", name="mst")
                for q in range(4):
                    cq = slice(co + 512 * q, co + 512 * (q + 1))
                    # bqe first on the PE so the scalar queue's e64 act
                    # never stalls waiting for it
                    bqe = pbc.tile([64, 512], F32, tag="bqe", bufs=1,
                                   name="bqe")
                    nc.tensor.matmul(bqe[:], sel8_sb[0:8, cm:cm + 64],
                                     Lrow[0:8, cq],
                                     start=True, stop=True)
                    bqm = pbc.tile([128, 512], F32, tag="bqm", name="bqm")
                    nc.tensor.matmul(bqm[:], sel8_sb[0:8, cm:cm + 128],
                                     Lrow[0:8, cq],
                                     start=True, stop=False)
                    nc.tensor.matmul(bqm[:], bfm_hl[0:48, sl],
                                     i16e_sb[0:48, 512 * q:512 * (q + 1)],
                                     start=False, stop=True)
                    e64 = psc.tile([64, 512], BF16, tag="e64", bufs=2,
                                   name="e64")
                    nc.scalar.activation(e64[:], bqe[:], AF.Exp)
                    nc.scalar.activation(mexp[:, 4 * q:4 * (q + 1)], bqm[:],
                                         AF.Exp)
                    # state-term factors: mst[s,(h,m)] = C[s,m] * exp(L[h,m])
                    nc.vector.tensor_tensor(
                        out=mst[:, 4 * q:4 * (q + 1)],
                        in0=C_t[:, sl].unsqueeze(1).broadcast_to([64, 4, 128]),
                        in1=e64.rearrange("p (h q2) -> p h q2", h=4),
                        op=OP.mult)
                stm = psc.tile([128, 16, 128], BF16, tag="stm", name="stm")
                nc.vector.scalar_tensor_tensor(
                    out=stm[:], in0=mexp[:], scalar=FMAX,
                    in1=St_all[c][:].unsqueeze(1).broadcast_to([128, 16, 128]),
                    op0=OP.min, op1=OP.mult)
                # w-scaled x for the chunk state summary
                xw = psc.tile([128, 16, 64], BF16, tag="xw", name="xw")
                nc.vector.tensor_tensor(
                    out=xw[:], in0=xtk.rearrange("p (h q2) -> p h q2", h=16),
                    in1=w_all[:, 16 * c:16 * (c + 1)].unsqueeze(2).broadcast_to(
                        [128, 16, 64]),
                    op=OP.mult)
                for hp in range(8):
                    h0, h1 = 2 * hp, 2 * hp + 1
                    yp = pbc.tile([128, 128], F32, tag="yp", name="yp")
                    nc.tensor.matmul(yp[0:64, :], hTb[:, 64 * h0:64 * h0 + 64],
                                     mst[:, h0], start=True, stop=False)
                    nc.tensor.matmul(yp[0:64, :], xtk[:, 64 * h0:64 * h0 + 64],
                                     stm[:, h0], start=False, stop=True)
                    nc.tensor.matmul(yp[64:128, :], hTb[:, 64 * h1:64 * h1 + 64],
                                     mst[:, h1], start=True, stop=False)
                    nc.tensor.matmul(yp[64:128, :], xtk[:, 64 * h1:64 * h1 + 64],
                                     stm[:, h1], start=False, stop=True)
                    nc.vector.scalar_tensor_tensor(
                        out=y_t[hp][:, sl], in0=xsil[hp][:, sl],
                        scalar=dcol_sb[:, l * 8 + hp:l * 8 + hp + 1],
                        in1=yp[:], op0=OP.mult, op1=OP.add)
                if PH == 51 and c == 0:
                    nc.vector.tensor_copy(x_res[0][:, 0:128], mexp[:, 0])
                    nc.vector.tensor_copy(x_res[0][:, 128:256], stm[:, 0])
                    nc.vector.tensor_copy(x_res[0][:, 256:384], mexp[:, 9])
                    nc.vector.tensor_copy(x_res[0][:, 384:512], stm[:, 9])
                    nc.vector.memset(x_res[1][:], 0.0)
                    nc.vector.tensor_copy(x_res[1][0:64, 0:128], mst[:, 0])
                    nc.vector.tensor_copy(x_res[1][0:64, 128:256], mst[:, 9])
                    nc.vector.tensor_copy(x_res[2][:], xtk[:])
                    nc.vector.tensor_copy(x_res[3][:],
                                          xw.rearrange("p h q2 -> p (h q2)"))
                # chunk state summary + decayed carry
                sS = psp.tile([64, 2, 512], F32, tag="sS", name="sS")
                xwf = xw.rearrange("p h q2 -> p (h q2)")
                nc.tensor.matmul(sS[:, 0], B_tok[c][:], xwf[:, 0:512],
                                 start=True, stop=True)
                nc.tensor.matmul(sS[:, 1], B_tok[c][:], xwf[:, 512:1024],
                                 start=True, stop=True)
                ht1 = psc.tile([64, 1024], F32, tag="ht1", bufs=1, name="ht1")
                nc.vector.tensor_tensor(
                    out=ht1.rearrange("p (h q2) -> p h q2", h=16),
                    in0=hT.rearrange("p (h q2) -> p h q2", h=16),
                    in1=gam_bc[:, 16 * c:16 * (c + 1)].unsqueeze(2).broadcast_to(
                        [64, 16, 64]),
                    op=OP.mult)
                nc.vector.tensor_tensor(out=hT[:], in0=ht1[:],
                                        in1=sS.rearrange("p a q2 -> p (a q2)"),
                                        op=OP.add)
            pxp.release()
            psp.release()
            pbc.release()
            psc.release()
        pB.release()
        pF2.release()
        if PH == 51:
            pCy.release()
            pCz.release()
            pF1.release()
            break
        if PH == 5:
            for i in range(4):
                nc.vector.tensor_copy(x_res[i][:], y_t[i][:])
        if PH == 55:
            for i in range(4):
                nc.vector.tensor_copy(x_res[i][:], y_t[4 + i][:])
        if PH <= 5 or PH == 55:
            pCy.release()
            pCz.release()
            pF1.release()
            break

        # ---------------- gating + rmsnorm (in place on y_t) ----------------
        with nc.named_scope(f"L{l}.gate"):
            pgt = tc.alloc_tile_pool(name=f"gt{l}", bufs=2, side="left")
            pgp = tc.alloc_tile_pool(name=f"gp{l}", bufs=2, space="PSUM")
            pgb = tc.alloc_tile_pool(name=f"gb{l}", bufs=2, space="PSUM")
            for f in range(8):
                nc.vector.tensor_tensor(out=y_t[f][:], in0=y_t[f][:],
                                        in1=z_t[f][:], op=OP.mult)
            r_rowr = pgt.tile([1, T], F32R, name="grrowr")
            for tb in range(2):
                sl = slice(512 * tb, 512 * (tb + 1))
                ps = pgp.tile([1, 512], F32, tag="gst", name="gst")
                for k in range(8):
                    g2 = pgt.tile([128, 512], BF16, tag="g2", bufs=3, name="g2")
                    nc.scalar.activation(g2[:], y_t[k][:, sl], AF.Square)
                    nc.tensor.matmul(ps[:], ones_c1, g2[:],
                                     start=(k == 0), stop=(k == 7))
                lnv = pgt.tile([1, 512], F32, tag="glnv", name="glnv")
                nc.scalar.activation(lnv[:], ps[:], AF.Ln, bias=eps1[:],
                                     scale=1.0 / D_INNER)
                nc.scalar.activation(r_rowr[0:1, sl], lnv[:], AF.Exp,
                                     scale=-0.5)
            for f in range(8):
                for tb in range(2):
                    sl = slice(512 * tb, 512 * (tb + 1))
                    rb = pgb.tile([128, 512], F32, tag="grb", name="grb")
                    nc.tensor.matmul(rb[:], ones_r[0:1, :], r_rowr[0:1, sl],
                                     start=True, stop=True)
                    nc.vector.scalar_tensor_tensor(
                        out=y_t[f][:, sl], in0=y_t[f][:, sl],
                        scalar=gwc_sb[:, l * 8 + f:l * 8 + f + 1], in1=rb[:],
                        op0=OP.mult, op1=OP.mult)
            pgb.release()
            pgp.release()
            pgt.release()
        pCz.release()
        if PH <= 6:
            pCy.release()
            pF1.release()
            break

        # ---------------- out_proj (+ residual) ----------------
        with nc.named_scope(f"L{l}.oproj"):
            pp = tc.alloc_tile_pool(name=f"opp{l}", bufs=3, space="PSUM")
            for mt in range(4):
                for tb in range(2):
                    sl = slice(512 * tb, 512 * (tb + 1))
                    ps = pp.tile([128, 512], F32, tag="mm", name="ps")
                    for k in range(8):
                        nc.tensor.matmul(ps[:], wopb[k][:, 128 * mt:128 * (mt + 1)],
                                         y_t[k][:, sl], start=(k == 0), stop=(k == 7))
                    nc.vector.tensor_tensor(out=x_res[mt][:, sl],
                                            in0=x_res[mt][:, sl], in1=ps[:],
                                            op=OP.add)
            pp.release()
        pCy.release()

        # ---------------- FFN ----------------
        pG = tc.alloc_tile_pool(name=f"G{l}", bufs=1, side="left")
        G_t = [pG.tile([128, T], BF16, tag=f"G{i}", name=f"G{i}")
               for i in range(16)]
        pH2 = tc.alloc_tile_pool(name=f"H2{l}", bufs=1, side="left")
        with nc.named_scope(f"L{l}.ln2"):
            h_ln2 = _ln(l, 1, pH2)
        with nc.named_scope(f"L{l}.ffn1"):
            pp = tc.alloc_tile_pool(name=f"f1p{l}", bufs=3, space="PSUM")
            for mt in range(16):
                for tb in range(2):
                    sl = slice(512 * tb, 512 * (tb + 1))
                    ps = pp.tile([128, 512], F32, tag="mm", name="ps")
                    for k in range(4):
                        nc.tensor.matmul(ps[:], wf1b[k][:, 128 * mt:128 * (mt + 1)],
                                         h_ln2[k][:, sl], start=(k == 0), stop=(k == 3))
                    nc.scalar.activation(G_t[mt][:, sl], ps[:], AF.Gelu_apprx_tanh,
                                         bias=bf1_sb[:, l * 16 + mt:l * 16 + mt + 1])
            pp.release()
        pH2.release()

        with nc.named_scope(f"L{l}.ffn2"):
            pp = tc.alloc_tile_pool(name=f"f2p{l}", bufs=3, space="PSUM")
            if l == 0 and PH >= 9:
                pcc = tc.alloc_tile_pool(name="ccsb", bufs=1, side="left")
                stg = pcc.tile([128, 4, T], BF16, name="ccstg")
            for mt in range(4):
                for tb in range(2):
                    sl = slice(512 * tb, 512 * (tb + 1))
                    ps = pp.tile([128, 512], F32, tag="mm", name="ps")
                    for k in range(16):
                        nc.tensor.matmul(ps[:], wf2b[k][:, 128 * mt:128 * (mt + 1)],
                                         G_t[k][:, sl], start=(k == 0), stop=(k == 15))
                    nc.vector.scalar_tensor_tensor(
                        out=x_res[mt][:, sl], in0=ps[:],
                        scalar=bf2_sb[:, l * 4 + mt:l * 4 + mt + 1],
                        in1=x_res[mt][:, sl], op0=OP.add, op1=OP.add)
                if l == 0 and PH >= 9:
                    nc.vector.tensor_copy(stg[:, mt], x_res[mt][:])
                    nc.sync.dma_start(cc_in[128 * mt:128 * (mt + 1), :], stg[:, mt])
                    nc.gpsimd.collective_compute(
                        "AllGather", OP.bypass,
                        ins=[cc_in[128 * mt:128 * (mt + 1), :]],
                        outs=[cc_out[mt].opt()],
                        replica_groups=[[0, 1], [2, 3], [4, 5], [6, 7]])
            pp.release()
        if not (l == 0 and PH >= 9):
            pG.release()
        pF1.release()
        if l == 1:
            pwB.release()

        # ---------------- pairwise combine after layer 0 ----------------
        if l == 0 and PH >= 9:
            # per-quarter readback+combine pipelined under later collectives
            with nc.named_scope("L0.comb"):
                for f in range(4):
                    cc_sb = pcc.tile([128, 2, T], BF16, tag=f"ccsb{f}",
                                     name=f"ccsb{f}")
                    for a in range(2):
                        nc.sync.dma_start(cc_sb[:, a], cc_out[f][a])
                    a_t = pcc.tile([128, T], BF16, tag=f"cca{f}",
                                   name=f"cca{f}")
                    for c in range(NCH):
                        nc.vector.tensor_tensor(
                            out=a_t[:, 128 * c:128 * (c + 1)],
                            in0=cc_sb[:, 0, 128 * c:128 * (c + 1)],
                            in1=_rev(cc_sb[:, 1,
                                     128 * (7 - c):128 * (8 - c)]),
                            op=OP.add)
                    for c in range(NCH):
                        tmp = pcc.tile([128, 128], F32, tag="cct", bufs=3,
                                       name="cct")
                        nc.vector.tensor_scalar(
                            out=tmp[:], in0=a_t[:, 128 * c:128 * (c + 1)],
                            scalar1=sel_t[:, 0:1], scalar2=None, op0=OP.mult)
                        nc.vector.scalar_tensor_tensor(
                            out=x_res[f][:, 128 * c:128 * (c + 1)],
                            in0=_rev(a_t[:, 128 * (7 - c):128 * (8 - c)]),
                            scalar=sel_t[:, 1:2], in1=tmp[:],
                            op0=OP.mult, op1=OP.add)
                pcc.release()
            pG.release()

    for i in range(4):
        nc.sync.dma_start(out_t.ap()[128 * i:128 * (i + 1), :], x_res[i][:])

    if pwA is not None:
        pwA.release()
    dram.release()
    const.release()


# ----------------------------------------------------------------------------
# host side
# ----------------------------------------------------------------------------

def _pos_enc():
    pos = np.arange(T, dtype=np.float32)[:, None]
    div = np.exp(-np.log(10000.0) * np.arange(0, D, 2, dtype=np.float32) / D)
    ang = pos * div
    return np.stack([np.sin(ang), np.cos(ang)], axis=-1).reshape(T, D)


def _shuffle_chunks(x_td):
    return np.ascontiguousarray(
        x_td.reshape(NCH, Q, *x_td.shape[1:])[::-1].reshape(x_td.shape))


def _core_inputs(inputs, b, d):
    f32 = np.float32
    x = np.asarray(inputs["x"], f32)[b] + _pos_enc()
    if d == 1:
        x = np.ascontiguousarray(x[::-1])
    im = {"x_fm": np.ascontiguousarray(x.T)}
    ls = [d, 2 + d]
    wip_ = np.zeros((2, D, MPAD), f32)
    for i, j in enumerate(ls):
        wip_[i, :, :D_INPROJ] = np.asarray(inputs["in_proj_w"], f32)[j]
    im["wip"] = np.ascontiguousarray(wip_.reshape(2, 4, 128, MPAD)).astype(BF)
    im["wop"] = np.ascontiguousarray(
        np.asarray(inputs["out_proj_w"], f32)[ls].reshape(2, 8, 128, D)).astype(BF)
    im["wf1"] = np.ascontiguousarray(
        np.asarray(inputs["ffn_w1"], f32)[ls].reshape(2, 4, 128, FFN)).astype(BF)
    im["wf2"] = np.ascontiguousarray(
        np.asarray(inputs["ffn_w2"], f32)[ls].reshape(2, 16, 128, D)).astype(BF)
    im["bf1"] = np.ascontiguousarray(
        np.asarray(inputs["ffn_b1"], f32)[ls].reshape(2, 16, 128).transpose(0, 2, 1))
    im["bf2"] = np.ascontiguousarray(
        np.asarray(inputs["ffn_b2"], f32)[ls].reshape(2, 4, 128).transpose(0, 2, 1))
    cw = np.asarray(inputs["conv_w"], f32)[ls]          # [2, 4, 1152]
    cw7 = np.zeros((2, 7, 1152), f32)
    cw7[:, 0:4] = cw
    im["wcv"] = np.ascontiguousarray(
        cw7.reshape(2, 7, 9, 128).transpose(0, 3, 2, 1).reshape(2, 128, 63))
    im["bcv"] = np.ascontiguousarray(
        np.asarray(inputs["conv_b"], f32)[ls].reshape(2, 9, 128).transpose(0, 2, 1))
    lnwa = np.stack([np.asarray(inputs["ln1_w"], f32)[ls],
                     np.asarray(inputs["ln2_w"], f32)[ls]], axis=1)
    im["lnw"] = np.ascontiguousarray(
        lnwa.reshape(2, 2, 4, 128).transpose(0, 1, 3, 2))
    im["gwc"] = np.ascontiguousarray(
        np.asarray(inputs["gnorm_w"], f32)[ls].reshape(2, 8, 128).transpose(0, 2, 1))
    Dp = np.asarray(inputs["Dparam"], f32)[ls]
    im["dcol"] = np.ascontiguousarray(
        np.repeat(Dp, 64, axis=1).reshape(2, 8, 128).transpose(0, 2, 1))
    dtb = np.zeros((32, 2), f32)
    dtb[:16] = np.asarray(inputs["dt_bias"], f32)[ls].T
    im["dtbt"] = dtb
    nea = np.zeros((32, 2), f32)
    nea[:16] = -np.exp(np.asarray(inputs["A_log"], f32)[ls]).T
    im["neat"] = nea
    jj, ii = np.meshgrid(np.arange(Q), np.arange(Q), indexing="ij")
    im["causal"] = (jj <= ii).astype(f32)
    im["identh"] = np.eye(128, dtype=f32).astype(BF)
    im["onesh"] = np.ones((1, 128), f32)
    im["onescol"] = np.ones((128, 1), f32)
    im["onesdh"] = np.ones((32, 128), f32)
    im["epsh"] = np.full((1, 1), EPS, f32)
    i16 = np.kron(np.eye(16, dtype=f32), np.ones((1, 128), f32))
    im["i16eh"] = np.concatenate([i16, np.zeros((16, 2048), f32), i16], axis=0)
    im["id32h"] = np.eye(32, dtype=f32)
    idsel = np.zeros((48, 32), f32)
    idsel[0:16, 0:16] = np.eye(16, dtype=f32)
    idsel[32:48, 0:16] = np.eye(16, dtype=f32)
    im["idselh"] = idsel
    sel8 = np.zeros((8, 512), f32)
    for j in range(4):
        sel8[2 * j:2 * j + 2, 128 * j:128 * (j + 1)] = 1.0
    im["sel8h"] = sel8
    sel = np.zeros((128, 2), f32)
    sel[:, 0 if d == 0 else 1] = 0.5
    im["selcol"] = sel
    return im


def _get_nc():
    if "nc" not in _CACHE:
        _CACHE["nc"] = build_nc()
    return _CACHE["nc"]


def kernel(**inputs):
    nc = _get_nc()
    in_maps = [_core_inputs(inputs, c // 2, c % 2) for c in range(8)]
    res = run_bass_kernel_spmd(nc, in_maps, list(range(8)))
    out = np.zeros((4, T, D), np.float32)
    for b in range(4):
        fwd = res.results[2 * b]["out_fm"].T
        bwd = np.ascontiguousarray(res.results[2 * b + 1]["out_fm"].T)[::-1]
        out[b] = 0.5 * (fwd + bwd)
    lengths = np.asarray(inputs["lengths"])
    mask = (np.arange(T)[None, :] < lengths[:, None]).astype(np.float32)
    return (out * mask[:, :, None]).astype(np.float32)


if __name__ == "__main__":
    print("building...")
    _get_nc()
    print("built ok")


# revision 45
# speedup vs baseline: 1.0013x; 1.0013x over previous
"""Bidirectional Vim-Mamba2 encoder on 8 Trainium2 NeuronCores.

Sharding: core c -> (sample b = c//2, direction d = c%2). Each core runs the
full per-sample block chain for its direction (chunked-SSD form of the
selective scan); the bidirectional combine after layer 1 is a pairwise
AllGather + on-chip average. Direction needs no data reversal on device:
bwd cores get chunk-reversed input (host side), anticausal masks / suffix
cumsums via per-core constant data, and a dynamic conv tap offset.

All big GEMMs run bf16 x bf16 (weights converted host-side, activations
produced in bf16); SSM state and residual stream stay fp32.

Scan decay broadcasts: per chunk the fp32 decay rows L[h, :] are flattened
h-major into a [1, 2048] row (one DMA) and broadcast across partitions by a
rank-1 f32r matmul; the per-token bias (-L_j + ln dt_j) rides a second
accumulating matmul against a block-select 0/1 constant, so exp() runs as
four wide activations per chunk with no per-head bias columns.

Self-contained: hardcodes shapes; imports concourse from known paths.
"""
import os
import sys

for _p in ("/opt/trn_rl_repo", "/root/.axon_site/_ro/trn_rl_repo"):
    if _p not in sys.path:
        sys.path.append(_p)

import numpy as np
import ml_dtypes
import concourse.bass as bass
import concourse.bacc as bacc
import concourse.mybir as mybir
from concourse import tile
from concourse.bass_utils import run_bass_kernel_spmd
from concourse.ap import AP


def _rev(ap, n=128):
    # reverse the innermost (contiguous) free axis of an AP
    a = [list(d) for d in ap.ap]
    assert a[-1][0] == 1 and a[-1][1] == n, a
    return AP(ap.tensor, ap.offset + n - 1, a[:-1] + [[-1, n]])

F32 = mybir.dt.float32
F32R = mybir.dt.float32r
BF16 = mybir.dt.bfloat16
I32 = mybir.dt.int32
AF = mybir.ActivationFunctionType
OP = mybir.AluOpType

T, D = 1024, 512
H, Q, NCH = 16, 128, 8
D_INNER = 1024
D_INPROJ = 2192
MPAD = 2304
FFN = 2048
EPS = 1e-5
FMAX = 3.0e38
BF = ml_dtypes.bfloat16

_CACHE = {}
PH = int(os.environ.get("KPH", "99"))


def build_nc():
    nc = bacc.Bacc("TRN2", target_bir_lowering=False, debug=False, num_devices=8)

    x_in = nc.dram_tensor("x_fm", [D, T], F32, kind="ExternalInput")
    wip = nc.dram_tensor("wip", [2, 4, 128, MPAD], BF16, kind="ExternalInput")
    wop = nc.dram_tensor("wop", [2, 8, 128, D], BF16, kind="ExternalInput")
    wf1 = nc.dram_tensor("wf1", [2, 4, 128, FFN], BF16, kind="ExternalInput")
    wf2 = nc.dram_tensor("wf2", [2, 16, 128, D], BF16, kind="ExternalInput")
    bf1 = nc.dram_tensor("bf1", [2, 128, 16], F32, kind="ExternalInput")
    bf2 = nc.dram_tensor("bf2", [2, 128, 4], F32, kind="ExternalInput")
    wcv = nc.dram_tensor("wcv", [2, 128, 63], F32, kind="ExternalInput")
    bcv = nc.dram_tensor("bcv", [2, 128, 9], F32, kind="ExternalInput")
    lnw = nc.dram_tensor("lnw", [2, 2, 128, 4], F32, kind="ExternalInput")
    gwc = nc.dram_tensor("gwc", [2, 128, 8], F32, kind="ExternalInput")
    dcol = nc.dram_tensor("dcol", [2, 128, 8], F32, kind="ExternalInput")
    dtbt = nc.dram_tensor("dtbt", [32, 2], F32, kind="ExternalInput")
    neat = nc.dram_tensor("neat", [32, 2], F32, kind="ExternalInput")
    causal = nc.dram_tensor("causal", [128, 128], F32, kind="ExternalInput")
    identh = nc.dram_tensor("identh", [128, 128], BF16, kind="ExternalInput")
    onesh = nc.dram_tensor("onesh", [1, 128], F32, kind="ExternalInput")
    onescol = nc.dram_tensor("onescol", [128, 1], F32, kind="ExternalInput")
    selcol = nc.dram_tensor("selcol", [128, 2], F32, kind="ExternalInput")
    onesdh = nc.dram_tensor("onesdh", [32, 128], F32, kind="ExternalInput")
    epsh = nc.dram_tensor("epsh", [1, 1], F32, kind="ExternalInput")
    i16eh = nc.dram_tensor("i16eh", [48, 2048], F32R, kind="ExternalInput")
    id32h = nc.dram_tensor("id32h", [32, 32], F32, kind="ExternalInput")
    idselh = nc.dram_tensor("idselh", [48, 32], F32R, kind="ExternalInput")
    sel8h = nc.dram_tensor("sel8h", [8, 512], F32R, kind="ExternalInput")
    out_t = nc.dram_tensor("out_fm", [D, T], F32, kind="ExternalOutput")

    with tile.TileContext(nc) as tc:
        _emit(nc, tc, locals())
    nc.compile()
    return nc


def _emit(nc, tc, t_):
    x_in, out_t = t_["x_in"], t_["out_t"]
    wip, wop, wf1, wf2 = t_["wip"], t_["wop"], t_["wf1"], t_["wf2"]

    const = tc.alloc_tile_pool(name="const", bufs=1)
    dram = tc.alloc_tile_pool(name="dram", bufs=1, space="DRAM")

    # ---------------- constants ----------------
    def load_const(name, shape, src_ap, dtype=F32):
        t = const.tile(shape, dtype, name=name)
        nc.sync.dma_start(t[:], src_ap)
        return t

    x_res = [const.tile([128, T], F32, name=f"xres{i}") for i in range(4)]
    for i in range(4):
        nc.sync.dma_start(x_res[i][:], x_in.ap()[128 * i:128 * (i + 1), :])

    causal_t = load_const("causal_t", [128, 128], t_["causal"].ap())
    ident_b = const.tile([128, 128], BF16, name="ident_b")
    nc.sync.dma_start(ident_b[:], t_["identh"].ap())
    ones_f = load_const("ones_f", [1, 128], t_["onesh"].ap())
    ones_r = const.tile([1, 128], F32R, name="ones_r")
    nc.vector.tensor_copy(ones_r[:], ones_f[:])
    onesP = const.tile([128, 128], F32R, name="onesP")
    nc.vector.memset(onesP.bitcast(F32), 1.0)
    onescol_f = load_const("onescol_f", [128, 1], t_["onescol"].ap())
    onescol_b = const.tile([128, 1], BF16, name="onescol_b")
    nc.vector.tensor_copy(onescol_b[:], onescol_f[:])
    i16e_sb = load_const("i16e_sb", [48, 2048], t_["i16eh"].ap(), dtype=F32R)
    id32f = load_const("id32f", [32, 32], t_["id32h"].ap())
    idsel = load_const("idsel", [48, 32], t_["idselh"].ap(), dtype=F32R)
    sel8_sb = load_const("sel8_sb", [8, 512], t_["sel8h"].ap(), dtype=F32R)
    dtb_t = load_const("dtb_t", [32, 2], t_["dtbt"].ap())
    nea_t = load_const("nea_t", [32, 2], t_["neat"].ap())
    lnw_sb = const.tile([128, 16], F32, name="lnw_sb")
    nc.sync.dma_start(lnw_sb.rearrange("p (a b c) -> p a b c", a=2, b=2),
                      t_["lnw"].ap().rearrange("a b p c -> p a b c"))
    gwc_sb = const.tile([128, 16], F32, name="gwc_sb")
    nc.sync.dma_start(gwc_sb.rearrange("p (a c) -> p a c", a=2),
                      t_["gwc"].ap().rearrange("a p c -> p a c"))
    dcol_sb = const.tile([128, 16], F32, name="dcol_sb")
    nc.sync.dma_start(dcol_sb.rearrange("p (a c) -> p a c", a=2),
                      t_["dcol"].ap().rearrange("a p c -> p a c"))
    wcv_sb = const.tile([128, 126], F32, name="wcv_sb")
    nc.sync.dma_start(wcv_sb.rearrange("p (a c) -> p a c", a=2),
                      t_["wcv"].ap().rearrange("a p c -> p a c"))
    bcv_sb = const.tile([128, 18], F32, name="bcv_sb")
    nc.sync.dma_start(bcv_sb.rearrange("p (a c) -> p a c", a=2),
                      t_["bcv"].ap().rearrange("a p c -> p a c"))
    bf1_sb = const.tile([128, 32], F32, name="bf1_sb")
    nc.sync.dma_start(bf1_sb.rearrange("p (a c) -> p a c", a=2),
                      t_["bf1"].ap().rearrange("a p c -> p a c"))
    bf2_sb = const.tile([128, 8], F32, name="bf2_sb")
    nc.sync.dma_start(bf2_sb.rearrange("p (a c) -> p a c", a=2),
                      t_["bf2"].ap().rearrange("a p c -> p a c"))
    sel_t = load_const("sel_t", [128, 2], t_["selcol"].ap())

    eps1 = load_const("eps1", [1, 1], t_["epsh"].ap())
    onesd_c = load_const("onesd_c", [32, 128], t_["onesdh"].ap())
    hT = const.tile([64, 1024], F32R, name="hT")

    # preloaded bf16 weights: layer 0 all + layer 1 in_proj at kernel start;
    # layer 1 out_proj/FFN staged at layer-1 start (SBUF headroom)
    pwA = tc.alloc_tile_pool(name="pwA", bufs=1, side="left")
    wopb0 = [pwA.tile([128, D], BF16, name=f"wopb0{k}") for k in range(8)]
    wf1b0 = [pwA.tile([128, FFN], BF16, name=f"wf1b0{k}") for k in range(4)]
    wf2b0 = [pwA.tile([128, D], BF16, name=f"wf2b0{k}") for k in range(16)]
    for k in range(8):
        nc.sync.dma_start(wopb0[k][:], wop.ap()[0, k])
    for k in range(4):
        nc.sync.dma_start(wf1b0[k][:], wf1.ap()[0, k])
    for k in range(16):
        nc.sync.dma_start(wf2b0[k][:], wf2.ap()[0, k])

    cc_in = dram.tile([D, T], BF16, name="cc_in")
    cc_out = [dram.tile([2, 128, T], BF16, name=f"cc_out{i}") for i in range(4)]

    ones_c1 = onescol_b[:, 0:1]   # [K=128, M=1] for partition-sum matmuls

    def _ln(l, which, out_pool):
        """feature-dim LayerNorm of x_res -> 4 bf16 tiles in out_pool."""
        w_col = lnw_sb[:, (l * 2 + which) * 4:(l * 2 + which) * 4 + 4]
        outs = [out_pool.tile([128, T], BF16, tag=f"hln{i}", name=f"hln{i}")
                for i in range(4)]
        px = tc.alloc_tile_pool(name=f"ln{l}{which}x", bufs=2)
        pp = tc.alloc_tile_pool(name=f"ln{l}{which}p", bufs=2, space="PSUM")
        xr, x2 = [], []
        for i in range(4):
            a = px.tile([128, T], BF16, tag="xr", bufs=4, name=f"xr{i}")
            nc.vector.tensor_copy(a[:], x_res[i][:])
            b = px.tile([128, T], BF16, tag="x2", bufs=4, name=f"x2{i}")
            nc.scalar.activation(b[:], x_res[i][:], AF.Square)
            xr.append(a)
            x2.append(b)
        m_row = px.tile([1, T], F32R, bufs=1, name="m_row")
        r_rowr = px.tile([1, T], F32R, bufs=1, name="r_rowr")
        for tb in range(2):
            sl = slice(512 * tb, 512 * (tb + 1))
            ps = pp.tile([1, 512], F32, tag="st", name="st")
            for k in range(4):
                nc.tensor.matmul(ps[:], ones_c1, xr[k][:, sl],
                                 start=(k == 0), stop=(k == 3))
            ps2 = pp.tile([1, 512], F32, tag="st2", name="st2")
            for k in range(4):
                nc.tensor.matmul(ps2[:], ones_c1, x2[k][:, sl],
                                 start=(k == 0), stop=(k == 3))
            nc.scalar.activation(m_row[0:1, sl], ps[:], AF.Copy, scale=1.0 / D)
            msq = px.tile([1, 512], F32, tag="msq", name="msq")
            nc.scalar.activation(msq[:], m_row[0:1, sl], AF.Square)
            var = px.tile([1, 512], F32, tag="var", name="var")
            nc.vector.scalar_tensor_tensor(out=var[:], in0=ps2[:],
                                           scalar=1.0 / D, in1=msq[:],
                                           op0=OP.mult, op1=OP.subtract)
            # 1/sqrt(v+eps) = exp(-0.5*ln(v+eps)): scalar-engine only, avoids
            # the slow DVE reciprocal (ln+exp share one act table set)
            lnv = px.tile([1, 512], F32, tag="lnv", name="lnv")
            nc.scalar.activation(lnv[:], var[:], AF.Ln, bias=eps1[:])
            nc.scalar.activation(r_rowr[0:1, sl], lnv[:], AF.Exp, scale=-0.5)
        for i in range(4):
            for tb in range(2):
                sl = slice(512 * tb, 512 * (tb + 1))
                mb = pp.tile([128, 512], F32, tag="mb", name="mb")
                nc.tensor.matmul(mb[:], ones_r[0:1, :], m_row[0:1, sl],
                                 start=True, stop=True)
                rb = pp.tile([128, 512], F32, tag="rb", name="rb")
                nc.tensor.matmul(rb[:], ones_r[0:1, :], r_rowr[0:1, sl],
                                 start=True, stop=True)
                tmp = px.tile([128, 512], F32, tag="tmp", name="tmp")
                nc.vector.tensor_tensor(out=tmp[:], in0=x_res[i][:, sl],
                                        in1=mb[:], op=OP.subtract)
                nc.vector.scalar_tensor_tensor(
                    out=outs[i][:, sl], in0=tmp[:], scalar=w_col[:, i:i + 1],
                    in1=rb[:], op0=OP.mult, op1=OP.mult)
        pp.release()
        px.release()
        return outs

    # ======================= layers =======================
    for l in range(1 if PH < 10 else 2):
        # layer-1 out_proj/FFN weights staged at layer start (DMA overlaps
        # the mixer phases)
        if l == 1:
            pwA.release()
            pwA = None
            pwB = tc.alloc_tile_pool(name="pwB", bufs=1, side="left")
            wopb = [pwB.tile([128, D], BF16, name=f"wopb1{k}")
                    for k in range(8)]
            wf1b = [pwB.tile([128, FFN], BF16, name=f"wf1b1{k}")
                    for k in range(4)]
            wf2b = [pwB.tile([128, D], BF16, name=f"wf2b1{k}")
                    for k in range(16)]
            for k in range(8):
                nc.sync.dma_start(wopb[k][:], wop.ap()[1, k])
            for k in range(4):
                nc.sync.dma_start(wf1b[k][:], wf1.ap()[1, k])
            for k in range(16):
                nc.sync.dma_start(wf2b[k][:], wf2.ap()[1, k])
        else:
            wopb, wf1b, wf2b = wopb0, wf1b0, wf2b0

        # two-sided stack pool management (LIFO per side)
        pCz = tc.alloc_tile_pool(name=f"Cz{l}", bufs=1, side="left")
        pF1 = tc.alloc_tile_pool(name=f"F1{l}", bufs=1, side="right")
        pA = tc.alloc_tile_pool(name=f"A{l}", bufs=1, side="right")
        # in_proj weights live on the right stack, released after in_proj
        # so the scan phase gets the SBUF back
        pwI = tc.alloc_tile_pool(name=f"pwI{l}", bufs=1, side="right")
        wipb = [pwI.tile([128, MPAD], BF16, name=f"wipb{l}{k}")
                for k in range(4)]
        for k in range(4):
            nc.sync.dma_start(wipb[k][:], wip.ap()[l, k])
        pH = tc.alloc_tile_pool(name=f"H{l}", bufs=1, side="left")

        z_t = [pCz.tile([128, T], BF16, tag=f"zt{i}", name=f"z{i}")
               for i in range(8)]
        xpad = [pA.tile([128, 1028], BF16, tag=f"xpad{f}", name=f"xpad{f}")
                for f in range(9)]
        dtr = pF1.tile([32, T], F32, name="dtr")
        nc.vector.memset(dtr[:], 0.0)

        if PH <= 0:
            pH.release()
            pwI.release()
            pA.release()
            pCz.release()
            pF1.release()
            break
        with nc.named_scope(f"L{l}.ln1"):
            h_ln1 = _ln(l, 0, pH)
        if PH <= 1:
            pH.release()
            pwI.release()
            pA.release()
            pCz.release()
            pF1.release()
            break

        # ---------------- in_proj ----------------
        with nc.named_scope(f"L{l}.inproj"):
            pp = tc.alloc_tile_pool(name=f"ipp{l}", bufs=3, space="PSUM")
            for mt in range(18):
                mo = 128 * mt
                for tb in range(2):
                    sl = slice(512 * tb, 512 * (tb + 1))
                    ps = pp.tile([128, 512], F32, tag="mm", name="ps")
                    for k in range(4):
                        nc.tensor.matmul(ps[:], wipb[k][:, mo:mo + 128],
                                         h_ln1[k][:, sl],
                                         start=(k == 0), stop=(k == 3))
                    if mt < 8:
                        nc.scalar.activation(z_t[mt][:, sl], ps[:], AF.Silu)
                    elif mt < 17:
                        f = mt - 8
                        nc.scalar.activation(
                            xpad[f][:, 3 + 512 * tb:3 + 512 * (tb + 1)], ps[:],
                            AF.Copy)
                    else:
                        nc.vector.tensor_copy(dtr[0:16, sl], ps[0:16, :])
            pp.release()
        pwI.release()
        pH.release()
        if PH <= 2:
            pA.release()
            pCz.release()
            pF1.release()
            break

        # ---------------- conv + silu (bf16) ----------------
        pB = tc.alloc_tile_pool(name=f"B{l}", bufs=1, side="left")
        xsil = [pB.tile([128, T], BF16, tag=f"xsil{f}", name=f"xsil{f}")
                for f in range(8)]
        B_t = pB.tile([64, T], BF16, name="B_t")
        C_t = pB.tile([64, T], BF16, name="C_t")
        with nc.named_scope(f"L{l}.conv"):
            # depthwise causal conv on the PE: stationary = diag(w_k) built
            # on the fly from the bf16 identity, moving = shifted x slices,
            # 4 taps accumulate in PSUM, silu reads PSUM
            pcv = tc.alloc_tile_pool(name=f"cv{l}", bufs=2, side="right")
            pcp = tc.alloc_tile_pool(name=f"cvp{l}", bufs=2, space="PSUM")
            dgs = []
            for f in range(9):
                nc.vector.memset(xpad[f][:, 0:3], 0.0)
                dg = pcv.tile([128, 4, 128], BF16, tag=f"dg{f}", name=f"dg{f}")
                for k in range(4):
                    w_k = wcv_sb[:, l * 63 + f * 7 + k:l * 63 + f * 7 + k + 1]
                    nc.vector.tensor_scalar(
                        out=dg[:, k], in0=ident_b[:, :], scalar1=w_k,
                        scalar2=None, op0=OP.mult)
                dgs.append(dg)
            # tb outer so the first half of every stream (incl. B/C) lands
            # early and the scan's first chunks can begin
            for tb in range(2):
                for f in range(9):
                    xp = xpad[f]
                    acc = pcp.tile([128, 512], F32, tag="acc", name="acc")
                    for k in range(4):
                        nc.tensor.matmul(
                            acc[:], dgs[f][:, k],
                            xp[:, k + 512 * tb:k + 512 * tb + 512],
                            start=(k == 0), stop=(k == 3))
                    sl2 = slice(512 * tb, 512 * (tb + 1))
                    if f < 8:
                        nc.scalar.activation(
                            xsil[f][:, sl2], acc[:], AF.Silu,
                            bias=bcv_sb[:, l * 9 + f:l * 9 + f + 1])
                    else:
                        nc.scalar.activation(
                            B_t[:, sl2], acc[0:64, :], AF.Silu,
                            bias=bcv_sb[0:64, l * 9 + f:l * 9 + f + 1])
                        nc.scalar.activation(
                            C_t[:, sl2], acc[64:128, :], AF.Silu,
                            bias=bcv_sb[64:128, l * 9 + f:l * 9 + f + 1])
            pcp.release()
            pcv.release()
        pA.release()
        if PH == 3:
            nc.vector.tensor_copy(x_res[0][:], xsil[0][:])
            nc.vector.tensor_copy(x_res[1][:], xsil[1][:])
            nc.vector.memset(x_res[2][:], 0.0)
            nc.vector.memset(x_res[3][:], 0.0)
            nc.vector.tensor_copy(x_res[2][0:64, :], B_t[:])
            nc.vector.tensor_copy(x_res[3][0:64, :], C_t[:])
        if PH <= 3:
            pB.release()
            pCz.release()
            pF1.release()
            break

        # ---------------- dt / decay family, St, B_tok ----------------
        pCy = tc.alloc_tile_pool(name=f"Cy{l}", bufs=1, side="right")
        y_t = [pCy.tile([128, T], BF16, tag=f"yt{i}", name=f"y{i}")
               for i in range(8)]
        pF2 = tc.alloc_tile_pool(name=f"F2{l}", bufs=1, side="right")
        pF3 = tc.alloc_tile_pool(name=f"F3{l}", bufs=1, side="right")
        dt_t = pF3.tile([32, T], F32, name="dt_t")
        log_a = pF3.tile([32, T], F32, name="log_a")
        gam_bc = pF2.tile([64, NCH * 16], F32, name="gam_bc")
        w_all = pF2.tile([128, NCH * 16], BF16, name="w_all")
        bfm_all = pF2.tile([32, NCH * 128], F32, name="bfm_all")
        Lf = pF2.tile([32, T], F32, name="Lf")
        # f32r hi/lo pairs (rows 0:16 hi, 32:48 lo residual, 16:32 zeroed)
        # so PE matmuls reconstruct full fp32 through two-row sums
        Lhl = pF2.tile([64, T], F32R, name="Lhl")
        bfm_hl = pF2.tile([64, T], F32R, name="bfm_hl")
        nc.vector.memset(Lhl.bitcast(F32)[0:32, :], 0.0)
        nc.vector.memset(bfm_hl.bitcast(F32)[0:32, :], 0.0)
        # decay-row broadcast staging: 8 chunk slots of (hi, lo) partition
        # pairs at bases {0,32,64,96} x 2 column slots, filled in the dt
        # phase so the scan's matmuls never wait on the sync queue
        Lrow = pF2.tile([128, 4096], F32R, name="Lrow")

        St_all = [pF2.tile([128, 128], BF16, tag=f"st{c}", name=f"St{c}")
                  for c in range(NCH)]
        B_tok = [pF2.tile([128, 64], BF16, tag=f"bt{c}", name=f"Bt{c}")
                 for c in range(NCH)]

        with nc.named_scope(f"L{l}.dtfam"):
            e_sp = pF3.tile([32, T], F32, name="e_sp")
            nc.scalar.activation(e_sp[:], dtr[:], AF.Exp, bias=dtb_t[:, l:l + 1])
            nc.scalar.activation(dt_t[:], e_sp[:], AF.Ln, bias=1.0)
            nc.vector.tensor_scalar(out=log_a[:], in0=dt_t[:],
                                    scalar1=nea_t[:, l:l + 1], scalar2=None,
                                    op0=OP.mult)

            pd = tc.alloc_tile_pool(name=f"dtf{l}", bufs=4, side="right")
            pg = tc.alloc_tile_pool(name=f"dtp{l}", bufs=2, space="PSUM")
            for c in range(NCH):
                sl = slice(Q * c, Q * (c + 1))
                la = log_a[:, sl]
                nc.vector.tensor_tensor_scan(Lf[:, sl], onesd_c[:], la, 0.0,
                                             OP.mult, OP.add)
                lndt = pd.tile([32, 128], F32, tag="lndt", name="lndt")
                nc.scalar.activation(lndt[:], dt_t[:, sl], AF.Ln)
                nc.vector.tensor_tensor(out=bfm_all[:, sl], in0=lndt[:],
                                        in1=Lf[:, sl], op=OP.subtract)
                # f32r hi/lo splits (rounding copy + residual)
                nc.vector.tensor_copy(Lhl[0:16, sl], Lf[0:16, sl])
                nc.vector.tensor_tensor(out=Lhl[32:48, sl], in0=Lf[0:16, sl],
                                        in1=Lhl[0:16, sl], op=OP.subtract)
                nc.vector.tensor_copy(bfm_hl[0:16, sl], bfm_all[0:16, sl])
                nc.vector.tensor_tensor(out=bfm_hl[32:48, sl],
                                        in0=bfm_all[0:16, sl],
                                        in1=bfm_hl[0:16, sl], op=OP.subtract)
                # flatten this chunk's L hi/lo rows h-major into its Lrow slot
                pb = 2 * (c % 4)
                co = 2048 * (c // 4)
                nc.sync.dma_start(
                    Lrow[pb:pb + 1, co:co + 2048].rearrange(
                        "p (h m) -> p h m", h=16), Lhl[0:16, sl])
                nc.sync.dma_start(
                    Lrow[pb + 1:pb + 2, co:co + 2048].rearrange(
                        "p (h m) -> p h m", h=16), Lhl[32:48, sl])
                # Lq (chunk-total log decay per head): select L[:, last token]
                # as a row via a tiny PE matmul (hi+lo summed by idsel)
                lqp = pg.tile([1, 32], F32, tag="lqp", name="lqp")
                nc.tensor.matmul(lqp[:], Lhl[0:48, Q * c + 127:Q * c + 128],
                                 idsel[:], start=True, stop=True)
                lqs = pd.tile([1, 32], F32, tag="lqs", name="lqs")
                nc.vector.tensor_copy(lqs[:], lqp[:])
                lqg = pd.tile([128, 16], F32, tag="lqg", name="lqg")
                nc.gpsimd.partition_broadcast(lqg[:], lqs[0:1, 0:16])
                nc.scalar.activation(gam_bc[:, 16 * c:16 * (c + 1)],
                                     lqg[0:64, :], AF.Exp)
                # w[j,h] = exp(Lq_h - L_j + lndt_j); bias columns via PE
                # transpose of bfm
                wtp = pg.tile([128, 32], F32, tag="wtp", name="wtp")
                nc.tensor.transpose(wtp[:], bfm_all[:, sl], id32f[:])
                wpre = pd.tile([128, 16], F32, tag="wpre", name="wpre")
                nc.vector.tensor_tensor(out=wpre[:], in0=wtp[:, 0:16],
                                        in1=lqg[:], op=OP.add)
                nc.scalar.activation(w_all[:, 16 * c:16 * (c + 1)], wpre[:],
                                     AF.Exp)
                stp = pg.tile([128, 128], F32, tag="stp", name="stp")
                nc.tensor.matmul(stp[:], B_t[:, sl], C_t[:, sl],
                                 start=True, stop=True)
                nc.vector.tensor_tensor(out=St_all[c][:], in0=stp[:],
                                        in1=causal_t[:], op=OP.mult)
                btp = pg.tile([128, 64], BF16, tag="btp", name="btp")
                nc.tensor.transpose(btp[:], B_t[:, sl], ident_b[0:64, 0:64])
                nc.vector.tensor_copy(B_tok[c][:], btp[:])
            pg.release()
            pd.release()
        pF3.release()
        if PH == 4:
            nc.vector.memset(x_res[0][:], 0.0)
            nc.vector.memset(x_res[1][:], 0.0)
            nc.vector.memset(x_res[3][:], 0.0)
            nc.vector.tensor_copy(x_res[0][0:32, :], Lf[:])
            nc.vector.tensor_copy(x_res[1][0:64, 0:128], gam_bc[:])
            nc.vector.tensor_copy(x_res[1][:, 128:256], w_all[:])
            nc.vector.tensor_copy(x_res[2][0:32, :], dt_t[:])
            for cc in range(8):
                nc.vector.tensor_copy(x_res[3][:, 128 * cc:128 * (cc + 1)],
                                      St_all[cc][:])
        if PH <= 4:
            pF2.release()
            pCy.release()
            pB.release()
            pCz.release()
            pF1.release()
            break

        # ---------------- scan ----------------
        nc.vector.memset(hT.bitcast(F32), 0.0)
        with nc.named_scope(f"L{l}.scan"):
            psc = tc.alloc_tile_pool(name=f"sc{l}", bufs=2, side="left")
            pbc = tc.alloc_tile_pool(name=f"bcp{l}", bufs=2, space="PSUM")
            psp = tc.alloc_tile_pool(name=f"spp{l}", bufs=1, space="PSUM")
            pxp = tc.alloc_tile_pool(name=f"xtp{l}", bufs=1, space="PSUM")
            for c in range(NCH):
                sl = slice(Q * c, Q * (c + 1))
                cm = 128 * (c % 4)
                co = 2048 * (c // 4)
                # x transposed: [token, (head, p)] in bf16
                xps = pxp.tile([128, 1024], BF16, tag="xps", name="xps")
                for f in range(8):
                    nc.tensor.transpose(xps[:, 128 * f:128 * (f + 1)],
                                        xsil[f][:, sl], ident_b[:, :])
                xtk = psc.tile([128, 1024], BF16, tag="xtk", name="xtk")
                nc.scalar.activation(xtk[:], xps[:], AF.Copy)
                hTb = psc.tile([64, 1024], BF16, tag="hTb", bufs=1, name="hTb")
                nc.scalar.activation(hTb[:], hT[:], AF.Copy)
                # within-chunk kernel: mexp[j,(h,m)] = exp(L[h,m] - L[h,j]
                # + lndt[h,j]); the L broadcast and the per-token bias both
                # ride the PE in fp32 (f32r hi/lo), so exp runs as 4 wide acts
                mexp = psc.tile([128, 16, 128], BF16, tag="mexp", name="mexp")
                mst = psc.tile([64, 16, 128], BF16, tag="mst", name="mst")
                for q in range(4):
                    cq = slice(co + 512 * q, co + 512 * (q + 1))
                    # bqe first on the PE so the scalar queue's e64 act
                    # never stalls waiting for it
                    bqe = pbc.tile([64, 512], F32, tag="bqe", bufs=1,
                                   name="bqe")
                    nc.tensor.matmul(bqe[:], sel8_sb[0:8, cm:cm + 64],
                                     Lrow[0:8, cq],
                                     start=True, stop=True)
                    bqm = pbc.tile([128, 512], F32, tag="bqm", name="bqm")
                    nc.tensor.matmul(bqm[:], sel8_sb[0:8, cm:cm + 128],
                                     Lrow[0:8, cq],
                                     start=True, stop=False)
                    nc.tensor.matmul(bqm[:], bfm_hl[0:48, sl],
                                     i16e_sb[0:48, 512 * q:512 * (q + 1)],
                                     start=False, stop=True)
                    e64 = psc.tile([64, 512], BF16, tag="e64", bufs=2,
                                   name="e64")
                    nc.scalar.activation(e64[:], bqe[:], AF.Exp)
                    nc.scalar.activation(mexp[:, 4 * q:4 * (q + 1)], bqm[:],
                                         AF.Exp)
                    # state-term factors: mst[s,(h,m)] = C[s,m] * exp(L[h,m])
                    nc.vector.tensor_tensor(
                        out=mst[:, 4 * q:4 * (q + 1)],
                        in0=C_t[:, sl].unsqueeze(1).broadcast_to([64, 4, 128]),
                        in1=e64.rearrange("p (h q2) -> p h q2", h=4),
                        op=OP.mult)
                stm = psc.tile([128, 16, 128], BF16, tag="stm", name="stm")
                nc.vector.scalar_tensor_tensor(
                    out=stm[:], in0=mexp[:], scalar=FMAX,
                    in1=St_all[c][:].unsqueeze(1).broadcast_to([128, 16, 128]),
                    op0=OP.min, op1=OP.mult)
                # w-scaled x for the chunk state summary
                xw = psc.tile([128, 16, 64], BF16, tag="xw", name="xw")
                nc.vector.tensor_tensor(
                    out=xw[:], in0=xtk.rearrange("p (h q2) -> p h q2", h=16),
                    in1=w_all[:, 16 * c:16 * (c + 1)].unsqueeze(2).broadcast_to(
                        [128, 16, 64]),
                    op=OP.mult)
                for hp in range(8):
                    h0, h1 = 2 * hp, 2 * hp + 1
                    yp = pbc.tile([128, 128], F32, tag="yp", name="yp")
                    nc.tensor.matmul(yp[0:64, :], hTb[:, 64 * h0:64 * h0 + 64],
                                     mst[:, h0], start=True, stop=False)
                    nc.tensor.matmul(yp[0:64, :], xtk[:, 64 * h0:64 * h0 + 64],
                                     stm[:, h0], start=False, stop=True)
                    nc.tensor.matmul(yp[64:128, :], hTb[:, 64 * h1:64 * h1 + 64],
                                     mst[:, h1], start=True, stop=False)
                    nc.tensor.matmul(yp[64:128, :], xtk[:, 64 * h1:64 * h1 + 64],
                                     stm[:, h1], start=False, stop=True)
                    nc.vector.scalar_tensor_tensor(
                        out=y_t[hp][:, sl], in0=xsil[hp][:, sl],
                        scalar=dcol_sb[:, l * 8 + hp:l * 8 + hp + 1],
                        in1=yp[:], op0=OP.mult, op1=OP.add)
                if PH == 51 and c == 0:
                    nc.vector.tensor_copy(x_res[0][:, 0:128], mexp[:, 0])
                    nc.vector.tensor_copy(x_res[0][:, 128:256], stm[:, 0])
                    nc.vector.tensor_copy(x_res[0][:, 256:384], mexp[:, 9])
                    nc.vector.tensor_copy(x_res[0][:, 384:512], stm[:, 9])
                    nc.vector.memset(x_res[1][:], 0.0)
                    nc.vector.tensor_copy(x_res[1][0:64, 0:128], mst[:, 0])
                    nc.vector.tensor_copy(x_res[1][0:64, 128:256], mst[:, 9])
                    nc.vector.tensor_copy(x_res[2][:], xtk[:])
                    nc.vector.tensor_copy(x_res[3][:],
                                          xw.rearrange("p h q2 -> p (h q2)"))
                # chunk state summary + decayed carry
                sS = psp.tile([64, 2, 512], F32, tag="sS", name="sS")
                xwf = xw.rearrange("p h q2 -> p (h q2)")
                nc.tensor.matmul(sS[:, 0], B_tok[c][:], xwf[:, 0:512],
                                 start=True, stop=True)
                nc.tensor.matmul(sS[:, 1], B_tok[c][:], xwf[:, 512:1024],
                                 start=True, stop=True)
                ht1 = psc.tile([64, 1024], F32, tag="ht1", bufs=1, name="ht1")
                nc.vector.tensor_tensor(
                    out=ht1.rearrange("p (h q2) -> p h q2", h=16),
                    in0=hT.rearrange("p (h q2) -> p h q2", h=16),
                    in1=gam_bc[:, 16 * c:16 * (c + 1)].unsqueeze(2).broadcast_to(
                        [64, 16, 64]),
                    op=OP.mult)
                nc.vector.tensor_tensor(out=hT[:], in0=ht1[:],
                                        in1=sS.rearrange("p a q2 -> p (a q2)"),
                                        op=OP.add)
            pxp.release()
            psp.release()
            pbc.release()
            psc.release()
        pB.release()
        pF2.release()
        if PH == 51:
            pCy.release()
            pCz.release()
            pF1.release()
            break
        if PH == 5:
            for i in range(4):
                nc.vector.tensor_copy(x_res[i][:], y_t[i][:])
        if PH == 55:
            for i in range(4):
                nc.vector.tensor_copy(x_res[i][:], y_t[4 + i][:])
        if PH <= 5 or PH == 55:
            pCy.release()
            pCz.release()
            pF1.release()
            break

        # ---------------- gating + rmsnorm (in place on y_t) ----------------
        with nc.named_scope(f"L{l}.gate"):
            pgt = tc.alloc_tile_pool(name=f"gt{l}", bufs=2, side="left")
            pgp = tc.alloc_tile_pool(name=f"gp{l}", bufs=2, space="PSUM")
            pgb = tc.alloc_tile_pool(name=f"gb{l}", bufs=2, space="PSUM")
            for f in range(8):
                nc.vector.tensor_tensor(out=y_t[f][:], in0=y_t[f][:],
                                        in1=z_t[f][:], op=OP.mult)
            r_rowr = pgt.tile([1, T], F32R, name="grrowr")
            for tb in range(2):
                sl = slice(512 * tb, 512 * (tb + 1))
                ps = pgp.tile([1, 512], F32, tag="gst", name="gst")
                for k in range(8):
                    g2 = pgt.tile([128, 512], BF16, tag="g2", bufs=3, name="g2")
                    nc.scalar.activation(g2[:], y_t[k][:, sl], AF.Square)
                    nc.tensor.matmul(ps[:], ones_c1, g2[:],
                                     start=(k == 0), stop=(k == 7))
                lnv = pgt.tile([1, 512], F32, tag="glnv", name="glnv")
                nc.scalar.activation(lnv[:], ps[:], AF.Ln, bias=eps1[:],
                                     scale=1.0 / D_INNER)
                nc.scalar.activation(r_rowr[0:1, sl], lnv[:], AF.Exp,
                                     scale=-0.5)
            for f in range(8):
                for tb in range(2):
                    sl = slice(512 * tb, 512 * (tb + 1))
                    rb = pgb.tile([128, 512], F32, tag="grb", name="grb")
                    nc.tensor.matmul(rb[:], ones_r[0:1, :], r_rowr[0:1, sl],
                                     start=True, stop=True)
                    nc.vector.scalar_tensor_tensor(
                        out=y_t[f][:, sl], in0=y_t[f][:, sl],
                        scalar=gwc_sb[:, l * 8 + f:l * 8 + f + 1], in1=rb[:],
                        op0=OP.mult, op1=OP.mult)
            pgb.release()
            pgp.release()
            pgt.release()
        pCz.release()
        if PH <= 6:
            pCy.release()
            pF1.release()
            break

        # ---------------- out_proj (+ residual) ----------------
        with nc.named_scope(f"L{l}.oproj"):
            pp = tc.alloc_tile_pool(name=f"opp{l}", bufs=3, space="PSUM")
            for mt in range(4):
                for tb in range(2):
                    sl = slice(512 * tb, 512 * (tb + 1))
                    ps = pp.tile([128, 512], F32, tag="mm", name="ps")
                    for k in range(8):
                        nc.tensor.matmul(ps[:], wopb[k][:, 128 * mt:128 * (mt + 1)],
                                         y_t[k][:, sl], start=(k == 0), stop=(k == 7))
                    nc.vector.tensor_tensor(out=x_res[mt][:, sl],
                                            in0=x_res[mt][:, sl], in1=ps[:],
                                            op=OP.add)
            pp.release()
        pCy.release()

        # ---------------- FFN ----------------
        pG = tc.alloc_tile_pool(name=f"G{l}", bufs=1, side="left")
        G_t = [pG.tile([128, T], BF16, tag=f"G{i}", name=f"G{i}")
               for i in range(16)]
        pH2 = tc.alloc_tile_pool(name=f"H2{l}", bufs=1, side="left")
        with nc.named_scope(f"L{l}.ln2"):
            h_ln2 = _ln(l, 1, pH2)
        with nc.named_scope(f"L{l}.ffn1"):
            pp = tc.alloc_tile_pool(name=f"f1p{l}", bufs=3, space="PSUM")
            for mt in range(16):
                for tb in range(2):
                    sl = slice(512 * tb, 512 * (tb + 1))
                    ps = pp.tile([128, 512], F32, tag="mm", name="ps")
                    for k in range(4):
                        nc.tensor.matmul(ps[:], wf1b[k][:, 128 * mt:128 * (mt + 1)],
                                         h_ln2[k][:, sl], start=(k == 0), stop=(k == 3))
                    nc.scalar.activation(G_t[mt][:, sl], ps[:], AF.Gelu_apprx_tanh,
                                         bias=bf1_sb[:, l * 16 + mt:l * 16 + mt + 1])
            pp.release()
        pH2.release()

        with nc.named_scope(f"L{l}.ffn2"):
            pp = tc.alloc_tile_pool(name=f"f2p{l}", bufs=3, space="PSUM")
            if l == 0 and PH >= 9:
                pcc = tc.alloc_tile_pool(name="ccsb", bufs=1, side="left")
                stg = pcc.tile([128, 4, T], BF16, name="ccstg")
            for mt in range(4):
                for tb in range(2):
                    sl = slice(512 * tb, 512 * (tb + 1))
                    ps = pp.tile([128, 512], F32, tag="mm", name="ps")
                    for k in range(16):
                        nc.tensor.matmul(ps[:], wf2b[k][:, 128 * mt:128 * (mt + 1)],
                                         G_t[k][:, sl], start=(k == 0), stop=(k == 15))
                    nc.vector.scalar_tensor_tensor(
                        out=x_res[mt][:, sl], in0=ps[:],
                        scalar=bf2_sb[:, l * 4 + mt:l * 4 + mt + 1],
                        in1=x_res[mt][:, sl], op0=OP.add, op1=OP.add)
                if l == 0 and PH >= 9:
                    nc.vector.tensor_copy(stg[:, mt], x_res[mt][:])
                    nc.sync.dma_start(cc_in[128 * mt:128 * (mt + 1), :], stg[:, mt])
                    nc.gpsimd.collective_compute(
                        "AllGather", OP.bypass,
                        ins=[cc_in[128 * mt:128 * (mt + 1), :]],
                        outs=[cc_out[mt].opt()],
                        replica_groups=[[0, 1], [2, 3], [4, 5], [6, 7]])
            pp.release()
        if not (l == 0 and PH >= 9):
            pG.release()
        pF1.release()
        if l == 1:
            pwB.release()

        # ---------------- pairwise combine after layer 0 ----------------
        if l == 0 and PH >= 9:
            # per-quarter readback+combine pipelined under later collectives
            with nc.named_scope("L0.comb"):
                for f in range(4):
                    cc_sb = pcc.tile([128, 2, T], BF16, tag=f"ccsb{f}",
                                     name=f"ccsb{f}")
                    for a in range(2):
                        nc.sync.dma_start(cc_sb[:, a], cc_out[f][a])
                    a_t = pcc.tile([128, T], BF16, tag=f"cca{f}",
                                   name=f"cca{f}")
                    for c in range(NCH):
                        nc.vector.tensor_tensor(
                            out=a_t[:, 128 * c:128 * (c + 1)],
                            in0=cc_sb[:, 0, 128 * c:128 * (c + 1)],
                            in1=_rev(cc_sb[:, 1,
                                     128 * (7 - c):128 * (8 - c)]),
                            op=OP.add)
                    for c in range(NCH):
                        tmp = pcc.tile([128, 128], F32, tag="cct", bufs=3,
                                       name="cct")
                        nc.vector.tensor_scalar(
                            out=tmp[:], in0=a_t[:, 128 * c:128 * (c + 1)],
                            scalar1=sel_t[:, 0:1], scalar2=None, op0=OP.mult)
                        nc.vector.scalar_tensor_tensor(
                            out=x_res[f][:, 128 * c:128 * (c + 1)],
                            in0=_rev(a_t[:, 128 * (7 - c):128 * (8 - c)]),
                            scalar=sel_t[:, 1:2], in1=tmp[:],
                            op0=OP.mult, op1=OP.add)
                pcc.release()
            pG.release()

    for i in range(4):
        nc.sync.dma_start(out_t.ap()[128 * i:128 * (i + 1), :], x_res[i][:])

    if pwA is not None:
        pwA.release()
    dram.release()
    const.release()


# ----------------------------------------------------------------------------
# host side
# ----------------------------------------------------------------------------

def _pos_enc():
    pos = np.arange(T, dtype=np.float32)[:, None]
    div = np.exp(-np.log(10000.0) * np.arange(0, D, 2, dtype=np.float32) / D)
    ang = pos * div
    return np.stack([np.sin(ang), np.cos(ang)], axis=-1).reshape(T, D)


def _shuffle_chunks(x_td):
    return np.ascontiguousarray(
        x_td.reshape(NCH, Q, *x_td.shape[1:])[::-1].reshape(x_td.shape))


def _core_inputs(inputs, b, d):
    f32 = np.float32
    x = np.asarray(inputs["x"], f32)[b] + _pos_enc()
    if d == 1:
        x = np.ascontiguousarray(x[::-1])
    im = {"x_fm": np.ascontiguousarray(x.T)}
    ls = [d, 2 + d]
    wip_ = np.zeros((2, D, MPAD), f32)
    for i, j in enumerate(ls):
        wip_[i, :, :D_INPROJ] = np.asarray(inputs["in_proj_w"], f32)[j]
    im["wip"] = np.ascontiguousarray(wip_.reshape(2, 4, 128, MPAD)).astype(BF)
    im["wop"] = np.ascontiguousarray(
        np.asarray(inputs["out_proj_w"], f32)[ls].reshape(2, 8, 128, D)).astype(BF)
    im["wf1"] = np.ascontiguousarray(
        np.asarray(inputs["ffn_w1"], f32)[ls].reshape(2, 4, 128, FFN)).astype(BF)
    im["wf2"] = np.ascontiguousarray(
        np.asarray(inputs["ffn_w2"], f32)[ls].reshape(2, 16, 128, D)).astype(BF)
    im["bf1"] = np.ascontiguousarray(
        np.asarray(inputs["ffn_b1"], f32)[ls].reshape(2, 16, 128).transpose(0, 2, 1))
    im["bf2"] = np.ascontiguousarray(
        np.asarray(inputs["ffn_b2"], f32)[ls].reshape(2, 4, 128).transpose(0, 2, 1))
    cw = np.asarray(inputs["conv_w"], f32)[ls]          # [2, 4, 1152]
    cw7 = np.zeros((2, 7, 1152), f32)
    cw7[:, 0:4] = cw
    im["wcv"] = np.ascontiguousarray(
        cw7.reshape(2, 7, 9, 128).transpose(0, 3, 2, 1).reshape(2, 128, 63))
    im["bcv"] = np.ascontiguousarray(
        np.asarray(inputs["conv_b"], f32)[ls].reshape(2, 9, 128).transpose(0, 2, 1))
    lnwa = np.stack([np.asarray(inputs["ln1_w"], f32)[ls],
                     np.asarray(inputs["ln2_w"], f32)[ls]], axis=1)
    im["lnw"] = np.ascontiguousarray(
        lnwa.reshape(2, 2, 4, 128).transpose(0, 1, 3, 2))
    im["gwc"] = np.ascontiguousarray(
        np.asarray(inputs["gnorm_w"], f32)[ls].reshape(2, 8, 128).transpose(0, 2, 1))
    Dp = np.asarray(inputs["Dparam"], f32)[ls]
    im["dcol"] = np.ascontiguousarray(
        np.repeat(Dp, 64, axis=1).reshape(2, 8, 128).transpose(0, 2, 1))
    dtb = np.zeros((32, 2), f32)
    dtb[:16] = np.asarray(inputs["dt_bias"], f32)[ls].T
    im["dtbt"] = dtb
    nea = np.zeros((32, 2), f32)
    nea[:16] = -np.exp(np.asarray(inputs["A_log"], f32)[ls]).T
    im["neat"] = nea
    jj, ii = np.meshgrid(np.arange(Q), np.arange(Q), indexing="ij")
    im["causal"] = (jj <= ii).astype(f32)
    im["identh"] = np.eye(128, dtype=f32).astype(BF)
    im["onesh"] = np.ones((1, 128), f32)
    im["onescol"] = np.ones((128, 1), f32)
    im["onesdh"] = np.ones((32, 128), f32)
    im["epsh"] = np.full((1, 1), EPS, f32)
    i16 = np.kron(np.eye(16, dtype=f32), np.ones((1, 128), f32))
    im["i16eh"] = np.concatenate([i16, np.zeros((16, 2048), f32), i16], axis=0)
    im["id32h"] = np.eye(32, dtype=f32)
    idsel = np.zeros((48, 32), f32)
    idsel[0:16, 0:16] = np.eye(16, dtype=f32)
    idsel[32:48, 0:16] = np.eye(16, dtype=f32)
    im["idselh"] = idsel
    sel8 = np.zeros((8, 512), f32)
    for j in range(4):
        sel8[2 * j:2 * j + 2, 128 * j:128 * (j + 1)] = 1.0
    im["sel8h"] = sel8
    sel = np.zeros((128, 2), f32)
    sel[:, 0 if d == 0 else 1] = 0.5
    im["selcol"] = sel
    return im


def _get_nc():
    if "nc" not in _CACHE:
        _CACHE["nc"] = build_nc()
    return _CACHE["nc"]


def kernel(**inputs):
    nc = _get_nc()
    in_maps = [_core_inputs(inputs, c // 2, c % 2) for c in range(8)]
    res = run_bass_kernel_spmd(nc, in_maps, list(range(8)))
    out = np.zeros((4, T, D), np.float32)
    for b in range(4):
        fwd = res.results[2 * b]["out_fm"].T
        bwd = np.ascontiguousarray(res.results[2 * b + 1]["out_fm"].T)[::-1]
        out[b] = 0.5 * (fwd + bwd)
    lengths = np.asarray(inputs["lengths"])
    mask = (np.arange(T)[None, :] < lengths[:, None]).astype(np.float32)
    return (out * mask[:, :, None]).astype(np.float32)


if __name__ == "__main__":
    print("building...")
    _get_nc()
    print("built ok")
